# revision 10
# baseline (speedup 1.0000x reference)
"""Trainium2 Bass kernel for nn_AttentionBlock (GroupNorm + single-head spatial
self-attention + residual), data-parallel over batch across 8 NeuronCores.

Reference per sample (C=256, H=W=32, N=H*W=1024 tokens, 32 groups):
    q = GN_q(x) @ Wq + bq ; k = GN_k(x) @ Wk + bk ; v = GN_v(x) @ Wv + bv
    att = softmax((q^T k) / sqrt(C)) over keys;  out = x + (att @ v^T) @ Wo + bo

v2 design (per core: 4 samples, software-pipelined):
  - Host folds GN affine + biases into weights (as v1): ONE score matmul
    chain via M^T = Wq_eff @ Wk_eff^T, output projection folded into the
    value weights (Wvo = Wv_eff @ Wo).  All attention matmuls fp8 DoubleRow.
  - Value/output bias (bvo) and out-proj bias (bo) ride the softmax
    row-sum=1 identity and fold into the host-prepared residual stream
    xbo = x^T + bo + bvo  (token-major bf16), which doubles as the residual
    add operand.
  - AV runs TRANSPOSED: out2[token, chan] = sum_m E[m,n] V[m,c], with e8
    slices as stationary weights.  Column sums accumulate as extra free=1
    matmuls against an SV-scaled ones vector into a small shared PSUM tile,
    so softmax denominators land per-PARTITION: the reciprocal is a tiny
    DVE op on [128,2] and the normalize+residual is ONE fused
    scalar_tensor_tensor per token tile (PSUM -> bf16 out):
        y^T[n, c] = out2[n, c] * rbc[n] + xbo[n, c]
  - Engine balance: ACT = 8 exps + 2 U evictions (Identity+w1 bias) + tiny
    GN rstd chain; DVE = bn_stats, xhat, V quantize, reciprocals, fused
    epilogue; PE = all matmuls (scores interleaved with the previous
    sample's AV so exp always has work); gpsimd/sync only launch DMAs.
  - Streams are bf16 (x, xbo, y); host transposes y^T back and upcasts.
  PSUM: 2x[128,1024] scores/U + 4x[128,512] AV-pairs/V/smalls = 8 banks.
"""

import numpy as np
import ml_dtypes

import concourse.bass as bass
import concourse.tile as tile
from concourse import mybir
from concourse.vector_clock import ScopedClock

F32 = mybir.dt.float32
BF16 = mybir.dt.bfloat16
FP8 = mybir.dt.float8e4
AF = mybir.ActivationFunctionType
ALU = mybir.AluOpType
DR = mybir.MatmulPerfMode.DoubleRow

B, C, H, W = 32, 256, 32, 32
N = H * W            # 1024 spatial tokens
G = 32               # groups
GS = C // G          # 8 channels per group
EPS = 1e-5
NCORES = 8
BS = B // NCORES     # 4 samples per core
CT = C // 128        # 2 channel partition-tiles
MT = N // 128        # 8 token partition-tiles
M2 = MT // 2         # 4 fp8-pair key tiles
SM = 256.0           # fp8 scale on the score chain (M, U)
SV = 32.0            # fp8 scale on the value chain (Wv, V)


def _patch_tile_drain():
    """walrus in this container allows only ONE sync wait per instruction;
    Tile's final drain carries one wait per live logical processor.  Split
    the waits across SP nops."""
    if getattr(tile.TileContext, "_drain_patched", False):
        return

    def _drain_and_barrier(self, tick_clock, wait_clock):
        nc = self.nc
        drain_inst = nc.sync.drain()
        wait_clock.add_sem_waits(
            drain_inst.ins, ScopedClock({None: tick_clock.global_clock})
        )
        si = drain_inst.ins.sync_info
        waits = list(si.on_wait or [])
        if len(waits) > 1:
            si.on_wait = waits[:1]
            for w in waits[1:]:
                nop_inst = nc.sync.nop()
                nop_inst.ins.sync_info = mybir.SyncInfo(on_wait=[w], on_update=[])

        nc.all_engine_barrier()
        assert self.sems is not None
        popped = nc._tile_sem_poison_stack.pop()
        assert popped is self._sem_poison
        nc.clear_and_free_semaphores(list(self.sems.allocated().values()))
        nc.all_engine_barrier()

    tile.TileContext._drain_and_barrier = _drain_and_barrier
    tile.TileContext._drain_patched = True


def _split_multi_waits(nc):
    """Hoist extra sync waits onto same-engine nops placed just before the
    instruction (engines execute their stream in order, so this is
    equivalent); walrus supports a single wait slot per instruction."""
    k = [0]
    for f in nc.m.functions:
        for b in f.blocks:
            insts = list(b.instructions)
            out = []
            changed = False
            for inst in insts:
                si = inst.sync_info
                if si is not None and si.on_wait and len(si.on_wait) > 1:
                    waits = list(si.on_wait)
                    for w in waits[:-1]:
                        nop = mybir.InstNoOp(
                            name=f"waitsplit-{k[0]}", ins=[], outs=[])
                        k[0] += 1
                        nop.engine = inst.engine
                        nop.sync_info = mybir.SyncInfo(
                            on_wait=[w], on_update=[])
                        out.append(nop)
                        nc.register_instruction(nop, overwrite=True)
                    si.on_wait = waits[-1:]
                    changed = True
                out.append(inst)
            if changed:
                lst = b.instructions
                lst.clear()
                lst.extend(out)
    return nc


def build_nc():
    _patch_tile_drain()
    nc = bass.Bass(trn_type="TRN2")

    x_d = nc.dram_tensor("x", [BS, CT, 128, N], BF16, kind="ExternalInput")
    xbo_d = nc.dram_tensor("xbo", [BS, MT, 128, C], BF16, kind="ExternalInput")
    y_d = nc.dram_tensor("y", [BS, MT, 128, C], BF16, kind="ExternalOutput")
    mt_d = nc.dram_tensor("mt", [128, 2, C], FP8, kind="ExternalInput")
    wv_d = nc.dram_tensor("wv", [128, 2, C], FP8, kind="ExternalInput")
    w1_d = nc.dram_tensor("w1", [CT, 128, 1], F32, kind="ExternalInput")
    ag_d = nc.dram_tensor("ag", [CT, 128, G], F32, kind="ExternalInput")
    bg_d = nc.dram_tensor("bg", [CT, G, 128], F32, kind="ExternalInput")

    with tile.TileContext(nc) as tc:
        _emit(nc, tc, x_d, xbo_d, y_d, mt_d, wv_d, w1_d, ag_d, bg_d)
    _split_multi_waits(nc)
    return nc


def _emit(nc, tc, x_d, xbo_d, y_d, mt_d, wv_d, w1_d, ag_d, bg_d):
    from contextlib import ExitStack
    ctx = ExitStack()
    with ctx:
        singles = ctx.enter_context(tc.tile_pool(name="singles", bufs=1))
        xpool = ctx.enter_context(tc.tile_pool(name="x", bufs=3))
        xbopool = ctx.enter_context(tc.tile_pool(name="xbo", bufs=3))
        stpool = ctx.enter_context(tc.tile_pool(name="st", bufs=2))
        xhpool = ctx.enter_context(tc.tile_pool(name="xh", bufs=2))
        upool = ctx.enter_context(tc.tile_pool(name="u", bufs=2))
        vpool = ctx.enter_context(tc.tile_pool(name="v", bufs=2))
        epool = ctx.enter_context(tc.tile_pool(name="e", bufs=2))
        rpool = ctx.enter_context(tc.tile_pool(name="r", bufs=2))
        opool = ctx.enter_context(tc.tile_pool(name="o", bufs=2))
        pps = ctx.enter_context(tc.tile_pool(name="pps", bufs=2, space="PSUM"))
        pou = ctx.enter_context(tc.tile_pool(name="pou", bufs=3, space="PSUM"))
        psm = ctx.enter_context(tc.tile_pool(name="psm", bufs=1, space="PSUM"))

        # ---- warm the ACT table (ln/exp set) while the first DMAs run ----
        eps_sb = singles.tile([128, 1], F32, tag="eps", name="eps")
        nc.vector.memset(eps_sb[:], EPS)
        actwarm = singles.tile([128, 1], F32, tag="actwarm", name="actwarm")
        nc.scalar.activation(actwarm[:], eps_sb[:], AF.Exp)
        nc.scalar.activation(actwarm[:], actwarm[:], AF.Ln)

        # one small psum tile, manually double-buffered by column region:
        # sample s uses columns [16*(s%2), 16*(s%2)+16):
        #   +0..8  colsum accumulators (cs)
        #   +8..10 group-stats combine (gps)
        #   +10..14 group mu/rstd broadcast (bc, per ct)
        smt = psm.tile([128, 32], F32, tag="smt", name="smt")

        x_sb = [None] * BS
        xbo_sb = [None] * BS
        xh8 = [None] * BS    # [128, 2, N] fp8 pair layout: c = 128j + p
        u8 = [None] * BS     # [128, 2, N] fp8 (score-chain, scaled by SM)
        v8 = [None] * BS     # 4x [128, 2, C] fp8 (value chain, scaled by SV)
        e8 = [None] * BS     # 4x [128, 2, N] fp8 exp(scores)
        gn_stats = [None] * BS

        def emit_load_x(s, spread=False):
            x_sb[s] = [xpool.tile([128, N], BF16, tag=f"x{t}", name=f"x{t}")
                       for t in range(CT)]
            for t in range(CT):
                if spread:
                    for h in range(2):
                        eng = nc.sync if h == 0 else nc.gpsimd
                        eng.dma_start(
                            x_sb[s][t][:, h * 512:(h + 1) * 512],
                            x_d[s, t, :, h * 512:(h + 1) * 512])
                else:
                    eng = nc.sync if t == 0 else nc.gpsimd
                    eng.dma_start(x_sb[s][t][:], x_d[s, t])

        def emit_load_xbo(s):
            xbo_sb[s] = xbopool.tile([128, MT, C], BF16, tag="xbo",
                                     name="xbo")
            for h in range(2):
                eng = nc.sync if h == 0 else nc.gpsimd
                eng.dma_start(
                    xbo_sb[s][:, h * 4:(h + 1) * 4, :],
                    xbo_d[s, h * 4:(h + 1) * 4].rearrange("m p c -> p m c"))

        # ---- constants / weights ----
        mt_sb = singles.tile([128, 2, C], FP8, tag="mt", name="mt")
        wv_sb = singles.tile([128, 2, C], FP8, tag="wv", name="wv")
        w1_sb = [singles.tile([128, 1], F32, tag=f"w1{t}", name=f"w1{t}")
                 for t in range(CT)]
        ag_sb = [singles.tile([128, G], F32, tag=f"ag{t}", name=f"ag{t}")
                 for t in range(CT)]
        bg_sb = [singles.tile([G, 128], F32, tag=f"bg{t}", name=f"bg{t}")
                 for t in range(CT)]
        nc.gpsimd.dma_start(mt_sb[:], mt_d[:, :, :])
        nc.gpsimd.dma_start(wv_sb[:], wv_d[:, :, :])
        for t in range(CT):
            nc.gpsimd.dma_start(w1_sb[t][:], w1_d[t])
            nc.sync.dma_start(ag_sb[t][:], ag_d[t])
            nc.sync.dma_start(bg_sb[t][:], bg_d[t])
        # SV-scaled ones: colsum matmuls then directly yield SV * sum(e),
        # whose reciprocal is the epilogue scale (SV unwind included).
        onesv = singles.tile([128, 2, 1], FP8, tag="onesv", name="onesv")
        nc.vector.memset(onesv[:], SV)

        def emit_gn_stats(s):
            # per-channel stats on DVE only (bf16 input)
            stats2 = []
            for t in range(CT):
                st6 = stpool.tile([128, 2, 6], F32, tag=f"st6_{t}",
                                  name=f"st6_{t}")
                for half in range(2):
                    nc.vector.bn_stats(
                        out=st6[:, half, :],
                        in_=x_sb[s][t][:, half * 512:(half + 1) * 512],
                    )
                aggr = stpool.tile([128, 2], F32, tag=f"aggr{t}",
                                   name=f"aggr{t}")
                nc.vector.bn_aggr(out=aggr[:], in_=st6[:])
                st2 = stpool.tile([128, 2], F32, tag=f"st2_{t}",
                                  name=f"st2_{t}")
                # (mu, msq = mu*mu + var) on the otherwise-idle gpsimd
                nc.gpsimd.tensor_copy(st2[:, 0:1], aggr[:, 0:1])
                nc.gpsimd.tensor_scalar(
                    out=st2[:, 1:2], in0=aggr[:, 0:1],
                    scalar1=aggr[:, 0:1], scalar2=aggr[:, 1:2],
                    op0=ALU.mult, op1=ALU.add,
                )
                stats2.append(st2)
            gn_stats[s] = stats2

        def emit_gn_combine_mm(s):
            # group combine on PE into the small shared psum tile
            base = 16 * (s % 2)
            stats2 = gn_stats[s]
            gps = smt[0:G, base + 8:base + 10]
            for t in range(CT):
                nc.tensor.matmul(gps, ag_sb[t][:], stats2[t][:],
                                 start=(t == 0), stop=(t == CT - 1))
            return gps

        def emit_gn_murs(s, gps):
            # group-level mu/rstd on 32 partitions; mu and -var read straight
            # from the combine psum (DVE smalls + ACT tinies)
            murs = stpool.tile([G, 2], F32, tag="murs", name="murs")
            nc.vector.tensor_copy(murs[:, 0:1], gps[:, 0:1])
            nv = stpool.tile([G, 1], F32, tag="nv", name="nv")
            nc.vector.tensor_scalar(
                out=nv[:], in0=gps[:, 0:1],
                scalar1=gps[:, 0:1], scalar2=gps[:, 1:2],
                op0=ALU.mult, op1=ALU.subtract)
            lnv = stpool.tile([G, 1], F32, tag="lnv", name="lnv")
            nc.scalar.activation(lnv[:], nv[:], AF.Ln,
                                 bias=eps_sb[0:G, :], scale=-1.0)
            nc.scalar.activation(murs[:, 1:2], lnv[:], AF.Exp, scale=-0.5)
            return murs

        def emit_gn_bcast_mm(s, murs, t):
            base = 16 * (s % 2)
            bcps = smt[:, base + 10 + 2 * t:base + 12 + 2 * t]
            nc.tensor.matmul(bcps, bg_sb[t][:], murs[:],
                             start=True, stop=True)
            return bcps

        def emit_xhat(s, t):
            base = 16 * (s % 2)
            mubc = stpool.tile([128, 2], F32, tag=f"mubc{t}",
                               name=f"mubc{t}")
            nc.vector.tensor_copy(mubc[:],
                                  smt[:, base + 10 + 2 * t:base + 12 + 2 * t])
            nc.vector.tensor_scalar(
                out=xh8[s][:, t, :], in0=x_sb[s][t][:],
                scalar1=mubc[:, 0:1], scalar2=mubc[:, 1:2],
                op0=ALU.subtract, op1=ALU.mult,
            )

        def emit_u_mm(s):
            # U' = M @ xh in PSUM (ps pool big slot)
            ups = []
            for ct in range(CT):
                ps = pps.tile([128, N], F32, tag="big", name="psu")
                for nch in range(2):
                    nc.tensor.matmul(
                        ps[:, nch * 512:(nch + 1) * 512],
                        mt_sb[:, :, ct * 128:(ct + 1) * 128],
                        xh8[s][:, :, nch * 512:(nch + 1) * 512],
                        start=True, stop=True, perf_mode=DR)
                ups.append(ps)
            return ups

        def emit_u_evict(s, ups, ct):
            # fp8 quantize + w1 bias on ACT
            nc.scalar.activation(
                u8[s][:, ct, :], ups[ct][:], AF.Identity,
                bias=w1_sb[ct][:])

        def emit_scores_mm(s, mt_):
            ps = pps.tile([128, N], F32, tag="big", name="pss")
            for nch in range(2):
                nc.tensor.matmul(
                    ps[:, nch * 512:(nch + 1) * 512],
                    xh8[s][:, :, mt_ * 128:(mt_ + 1) * 128],
                    u8[s][:, :, nch * 512:(nch + 1) * 512],
                    start=True, stop=True, perf_mode=DR)
            return ps

        def emit_exp(s, mt_, ps):
            nc.scalar.activation(e8[s][mt_ // 2][:, mt_ % 2, :], ps[:],
                                 AF.Exp, scale=1.0 / SM)

        def emit_v_mm(s, m2):
            ps = pou.tile([128, 2 * C], F32, tag="o2", name="psv")
            for j in range(2):
                nc.tensor.matmul(
                    ps[:, j * C:(j + 1) * C],
                    xh8[s][:, :, (2 * m2 + j) * 128:(2 * m2 + j + 1) * 128],
                    wv_sb[:],
                    start=True, stop=True, perf_mode=DR)
            return ps

        def emit_v_evict(s, m2, ps, on_act=False):
            # pure fp8 quantize (bias folded into xbo on the host)
            dst = v8[s][m2][:].rearrange("p a b -> p (a b)")
            if on_act:
                nc.scalar.activation(dst, ps[:], AF.Identity)
            else:
                nc.vector.tensor_scalar(
                    out=dst, in0=ps[:], scalar1=1.0, scalar2=None,
                    op0=ALU.mult)

        def alloc_sample(k):
            xh8[k] = xhpool.tile([128, 2, N], FP8, tag="xh8", name="xh8")
            u8[k] = upool.tile([128, 2, N], FP8, tag="u8", name="u8")

        def emit_stageA(k):
            # GN combine -> murs -> bcast -> xhat -> U -> Uev for sample k
            gps = emit_gn_combine_mm(k)
            murs = emit_gn_murs(k, gps)
            alloc_sample(k)
            for t in range(CT):
                emit_gn_bcast_mm(k, murs, t)
                emit_xhat(k, t)
            ups = emit_u_mm(k)
            for ct in range(CT):
                emit_u_evict(k, ups, ct)

        def alloc_e8(k):
            e8[k] = [epool.tile([128, 2, N], FP8, tag=f"e8_{m2}",
                                name=f"e8_{m2}") for m2 in range(M2)]

        def alloc_v8(k):
            v8[k] = [vpool.tile([128, 2, C], FP8, tag=f"v8_{m2}",
                                name=f"v8_{m2}") for m2 in range(M2)]

        # -------- prologue: produce sample 0 fully, stageA(1) --------
        emit_load_x(0, spread=True)
        emit_load_xbo(0)
        emit_load_x(1)
        emit_gn_stats(0)
        emit_stageA(0)
        alloc_e8(0)
        for mt_ in range(MT):
            ps = emit_scores_mm(0, mt_)
            emit_exp(0, mt_, ps)
        alloc_v8(0)
        emit_load_xbo(1)
        emit_gn_stats(1)
        for m2 in range(M2):
            psv = emit_v_mm(0, m2)
            emit_v_evict(0, m2, psv, on_act=(m2 == M2 - 1))
        emit_stageA(1)

        # -------- steady windows: AV(s) + scores/exp(s+1) + stageA(s+2)
        #          + V(s+1), exp chain paces the window --------
        for s in range(BS):
            nxt = s + 1 < BS
            nxt2 = s + 2 < BS
            base = 16 * (s % 2)
            cs = smt[:, base:base + 8]
            if nxt2:
                emit_load_x(s + 2)
                emit_load_xbo(s + 2)

            rbc = rpool.tile([128, 8], F32, tag="rbc", name="rbc")
            o_sb = opool.tile([128, MT, C], BF16, tag="o", name="o")
            if nxt:
                alloc_e8(s + 1)

            def score_pair(mt_):
                if nxt:
                    ps = emit_scores_mm(s + 1, mt_)
                    emit_exp(s + 1, mt_, ps)

            def av_pair(pair):
                o2 = pou.tile([128, 512], F32, tag="o2", name="o2")
                for half in range(2):
                    nt = pair * 2 + half
                    dst = o2[:, half * 256:(half + 1) * 256]
                    for m2 in range(M2):
                        lhs = e8[s][m2][:, :, nt * 128:(nt + 1) * 128]
                        nc.tensor.matmul(
                            dst, lhs, v8[s][m2][:],
                            start=(m2 == 0), stop=(m2 == M2 - 1),
                            perf_mode=DR)
                        nc.tensor.matmul(
                            cs[:, nt:nt + 1], lhs, onesv[:],
                            start=(m2 == 0), stop=(m2 == M2 - 1),
                            perf_mode=DR)
                return o2

            def epi(pair, o2):
                for half in range(2):
                    nt = pair * 2 + half
                    nc.vector.scalar_tensor_tensor(
                        out=o_sb[:, nt, :],
                        in0=o2[:, half * 256:(half + 1) * 256],
                        scalar=rbc[:, nt:nt + 1],
                        in1=xbo_sb[s][:, nt, :],
                        op0=ALU.mult, op1=ALU.add)

            score_pair(0)
            score_pair(1)
            o2a = av_pair(0)
            score_pair(2)
            if nxt2:
                # stats for s+2 start as soon as x arrives (DVE mid-window)
                emit_gn_stats(s + 2)
            o2b = av_pair(1)
            nc.vector.reciprocal(rbc[:, 0:4], cs[:, 0:4])
            epi(0, o2a)
            epi(1, o2b)
            score_pair(3)
            o2a = av_pair(2)
            score_pair(4)
            if nxt2:
                # GN chain for s+2 overlaps the exp chain (PE/DVE/ACT mid)
                gps = emit_gn_combine_mm(s + 2)
                murs = emit_gn_murs(s + 2, gps)
                alloc_sample(s + 2)
                for t in range(CT):
                    emit_gn_bcast_mm(s + 2, murs, t)
                    emit_xhat(s + 2, t)
            score_pair(5)
            o2b = av_pair(3)
            nc.vector.reciprocal(rbc[:, 4:8], cs[:, 4:8])
            epi(2, o2a)
            epi(3, o2b)
            if nxt2:
                ups = emit_u_mm(s + 2)
                emit_u_evict(s + 2, ups, 0)
            score_pair(6)
            if nxt2:
                emit_u_evict(s + 2, ups, 1)
            score_pair(7)
            nc.sync.dma_start(
                y_d[s, 0:4].rearrange("m p c -> p m c"), o_sb[:, 0:4, :])
            nc.gpsimd.dma_start(
                y_d[s, 4:8].rearrange("m p c -> p m c"), o_sb[:, 4:8, :])

            if nxt:
                alloc_v8(s + 1)
                for m2 in range(M2):
                    psv = emit_v_mm(s + 1, m2)
                    emit_v_evict(s + 1, m2, psv, on_act=(m2 == M2 - 1))


_NC_CACHE = {}


def _get_nc():
    if "nc" not in _NC_CACHE:
        _NC_CACHE["nc"] = build_nc()
    return _NC_CACHE["nc"]


def _pair(a):
    """[C, X] -> [128, 2, X] fp8 pair layout with c = 128*j + p."""
    a = np.asarray(a, np.float32)
    return np.ascontiguousarray(
        a.reshape(2, 128, a.shape[1]).transpose(1, 0, 2))


def _fp8(a):
    return np.clip(np.asarray(a, np.float32),
                   -240, 240).astype(ml_dtypes.float8_e4m3)


def make_in_maps(**inputs):
    f32 = np.float32
    bf = ml_dtypes.bfloat16
    x = np.asarray(inputs["x"], f32).reshape(B, C, N)
    Wq = np.asarray(inputs["Wq"], f32)
    Wk = np.asarray(inputs["Wk"], f32)
    Wv = np.asarray(inputs["Wv"], f32)
    Wo = np.asarray(inputs["Wo"], f32)
    bq = np.asarray(inputs["bq"], f32)
    bv = np.asarray(inputs["bv"], f32)
    bo = np.asarray(inputs["bo"], f32)
    gq_s = np.asarray(inputs["gq_s"], f32)
    gq_b = np.asarray(inputs["gq_b"], f32)
    gk_s = np.asarray(inputs["gk_s"], f32)
    gv_s = np.asarray(inputs["gv_s"], f32)
    gv_b = np.asarray(inputs["gv_b"], f32)
    # bk and gk_b only shift scores uniformly along the softmax axis -> cancel

    inv_sqrt_c = float(C) ** -0.5
    Wq_eff = (gq_s[:, None] * Wq) * inv_sqrt_c
    bq_eff = (gq_b @ Wq + bq) * inv_sqrt_c
    Wk_eff = gk_s[:, None] * Wk
    m_t = (Wq_eff @ Wk_eff.T) * SM       # lhsT for U: [c', c], fp8-scaled
    w1 = (Wk_eff @ bq_eff) * SM          # [c]
    Wv_eff = gv_s[:, None] * Wv
    bv_eff = gv_b @ Wv + bv
    # fold the output projection into the value chain; its bias (and bo)
    # ride softmax row-sum=1 into the residual stream
    Wvo = Wv_eff @ Wo
    bvo = bv_eff @ Wo
    badd = (bvo + bo).astype(f32)        # [C]

    ag = np.zeros((C, G), f32)
    bg = np.zeros((G, C), f32)
    for c in range(C):
        ag[c, c // GS] = 1.0 / GS
        bg[c // GS, c] = 1.0

    # token-major residual stream: xbo[b, n, c] = x[b, c, n] + badd[c]
    xbo = (x.transpose(0, 2, 1) + badd[None, None, :]).astype(bf)
    xbo = np.ascontiguousarray(xbo.reshape(B, MT, 128, C))

    shared = {
        "mt": _fp8(_pair(m_t)),
        "wv": _fp8(_pair(Wvo * SV)),
        "w1": w1.astype(f32).reshape(CT, 128, 1),
        "ag": np.ascontiguousarray(ag.reshape(CT, 128, G)),
        "bg": np.ascontiguousarray(bg.reshape(G, CT, 128).transpose(1, 0, 2)),
    }
    xbf = x.astype(bf).reshape(B, CT, 128, N)
    in_maps = []
    for i in range(NCORES):
        m = dict(shared)
        m["x"] = np.ascontiguousarray(xbf[i * BS:(i + 1) * BS])
        m["xbo"] = np.ascontiguousarray(xbo[i * BS:(i + 1) * BS])
        in_maps.append(m)
    return in_maps


def run_sharded(inputs, trace=False, **kwargs):
    from concourse.bass_utils import run_bass_kernel_spmd
    nc = _get_nc()
    in_maps = make_in_maps(**inputs)
    res = run_bass_kernel_spmd(nc, in_maps, core_ids=list(range(NCORES)),
                               trace=trace, **kwargs)
    outs = [np.asarray(res.results[i]["y"], ml_dtypes.bfloat16)
            for i in range(NCORES)]
    # y_dev[s, mt, p, c] = y^T: token-major; transpose back to [C, N]
    yt = np.concatenate(outs, axis=0).astype(np.float32)  # [B, MT, 128, C]
    full = yt.reshape(B, N, C).transpose(0, 2, 1).reshape(B, C, H, W)
    return np.ascontiguousarray(full), res


def kernel(**inputs):
    out, _ = run_sharded(inputs, trace=False)
    return out
